# revision 21
# baseline (speedup 1.0000x reference)
"""MoE (routed top-2 + shared expert) Trainium2 kernel, 8-core expert-parallel.

Distribution strategy (hardcoded for B=4,S=2048,H=1024,E=8,K=2,I=1024,NSH=2):
 - Host computes the router (gate logits / softmax / top-2 / capacity mask)
   with the same jax-on-CPU ops as the reference, then dispatches tokens:
   core c receives the tokens routed to expert c, gathered and transposed to
   [H, cap] bf16, plus per-slot combine weights.
 - Core c runs expert c's SwiGLU MLP on its token buffer (weights resident in
   SBUF, bf16 matmuls into fp32 PSUM; down-proj emits eo as [H, slots]).
 - The shared expert is token-parallel: core c runs the full shared SwiGLU on
   tokens [c*1024, (c+1)*1024), with all shared weights SBUF-resident.
 - Host gathers the expert outputs (applying the top-2 combine weights during
   the gather) and adds the shared output.

bf16 matmuls: ~1.2-1.4x the fp32r MM rate (16-bit moving stream + weight-load
overlap), l2 rel err ~4.4e-3 vs the 2e-2 gate. The routed token buffer is
trimmed at build time to the actual max expert load (padded to 128): 2176
slots instead of the 2560 capacity for the reference routing, saving ~15% of
the routed compute.
"""

import contextlib
import math

import numpy as np

import concourse.mybir as mybir
import concourse.tile as tile
from concourse import bacc
from concourse.bass_utils import run_bass_kernel_spmd

# Problem dims (hardcoded per spec)
B, S, H = 4, 2048, 1024
E, TOPK, I = 8, 2, 1024
NSH = 2
ISH = NSH * I            # 2048 shared intermediate
RSF = 1.0
N = B * S                # 8192 tokens
CAP = 2560               # ceil(1.25 * N * TOPK / E)
TSH = N // 8             # shared-expert tokens per core
P = 128
f32 = mybir.dt.float32
bf16 = mybir.dt.bfloat16
KH = H // P              # 8 contraction subtiles over H
KI = I // P              # 8 over I
KISH = ISH // P          # 16 over ISH
FD = 512                 # matmul moving free dim
Silu = mybir.ActivationFunctionType.Silu

# Active routed slots (padded to 128); set by kernel() from the actual max
# expert load before building. 2176 covers the near-uniform load of the
# reference distribution (max count ~2080); kernel() adjusts it at runtime.
NSLOT = 2176


def _chunks(total):
    """[(offset, width)] with width 512 except a possibly-smaller tail."""
    out = []
    off = 0
    while off < total:
        w = min(FD, total - off)
        out.append((off, w))
        off += w
    return out


def _groups(total):
    """Chunk groups of <=4 chunks (psum: 4 gate + 4 up banks)."""
    chs = _chunks(total)
    return [chs[i:i + 4] for i in range(0, len(chs), 4)]


def _declare(nc):
    t = {}
    t["xe_t"] = nc.dram_tensor("xe_t", [H, CAP], bf16, kind="ExternalInput")
    t["wg_t"] = nc.dram_tensor("wg_t", [H, I], bf16, kind="ExternalInput")
    t["wu_t"] = nc.dram_tensor("wu_t", [H, I], bf16, kind="ExternalInput")
    t["wd_t"] = nc.dram_tensor("wd_t", [I, H], bf16, kind="ExternalInput")
    t["xs_t"] = nc.dram_tensor("xs_t", [H, TSH], bf16, kind="ExternalInput")
    t["wsg_t"] = nc.dram_tensor("wsg_t", [H, ISH], bf16, kind="ExternalInput")
    t["wsu_t"] = nc.dram_tensor("wsu_t", [H, ISH], bf16, kind="ExternalInput")
    t["wsd_t"] = nc.dram_tensor("wsd_t", [ISH, H], bf16, kind="ExternalInput")
    t["eo"] = nc.dram_tensor("eo", [H, CAP], bf16, kind="ExternalOutput")
    t["so"] = nc.dram_tensor("so", [TSH, H], bf16, kind="ExternalOutput")
    _rearranges(t)
    return t


def _declare_internal(nc):
    """Same tensors as _declare but Internal DRAM — used by timing harnesses
    so per-call wall time carries no host<->device transfer of real data."""
    t = {}
    for name, shape, dt in [
            ("xe_t", [H, CAP], bf16), ("wg_t", [H, I], bf16),
            ("wu_t", [H, I], bf16), ("wd_t", [I, H], bf16),
            ("xs_t", [H, TSH], bf16),
            ("wsg_t", [H, ISH], bf16), ("wsu_t", [H, ISH], bf16),
            ("wsd_t", [ISH, H], bf16), ("eo", [H, CAP], bf16),
            ("so", [TSH, H], bf16)]:
        t[name] = nc.dram_tensor(name, shape, dt)
    _rearranges(t)
    return t


def _rearranges(t):
    t["xe_r"] = t["xe_t"][:].rearrange("(k p) t -> p k t", p=P)    # [128,8,2560]
    t["wg_r"] = t["wg_t"][:].rearrange("(k p) i -> p k i", p=P)    # [128,8,1024]
    t["wu_r"] = t["wu_t"][:].rearrange("(k p) i -> p k i", p=P)
    t["wd_r"] = t["wd_t"][:].rearrange("(k p) h -> p k h", p=P)
    t["xs_r"] = t["xs_t"][:].rearrange("(k p) t -> p k t", p=P)    # [128,8,1024]
    t["wsg_r"] = t["wsg_t"][:].rearrange("(k p) i -> p k i", p=P)  # [128,8,2048]
    t["wsu_r"] = t["wsu_t"][:].rearrange("(k p) i -> p k i", p=P)
    t["wsd_r"] = t["wsd_t"][:].rearrange("(k p) h -> p k h", p=P)  # [128,16,1024]


def _pools_routed(tc, ctx):
    return {
        "w": ctx.enter_context(tc.tile_pool(name="wR", bufs=1)),
        "x": ctx.enter_context(tc.tile_pool(name="xR", bufs=2)),
        "h": ctx.enter_context(tc.tile_pool(name="hR", bufs=2)),
        "t": ctx.enter_context(tc.tile_pool(name="tR", bufs=3)),
        "o": ctx.enter_context(tc.tile_pool(name="oR", bufs=4)),
    }


def _pools_shared(tc, ctx):
    return {
        "w": ctx.enter_context(tc.tile_pool(name="wS", bufs=1)),
        "gu": ctx.enter_context(tc.tile_pool(name="guS", bufs=2)),
        "d": ctx.enter_context(tc.tile_pool(name="dS", bufs=2)),
        "t": ctx.enter_context(tc.tile_pool(name="tS", bufs=3)),
        "o": ctx.enter_context(tc.tile_pool(name="oS", bufs=4)),
    }


def _emit_routed_weights(nc, t, pools):
    """Load expert weights resident in SBUF (once, outside any timing loop)."""
    w = pools["w"]
    wg_sb = w.tile([P, KH, I], bf16, tag="wg")
    wu_sb = w.tile([P, KH, I], bf16, tag="wu")
    wd_sb = w.tile([P, KI, H], bf16, tag="wd")
    for k in range(KH):
        nc.sync.dma_start(wg_sb[:, k], t["wg_r"][:, k])
        nc.sync.dma_start(wu_sb[:, k], t["wu_r"][:, k])
    for k in range(KI):
        nc.sync.dma_start(wd_sb[:, k], t["wd_r"][:, k])
    return wg_sb, wu_sb, wd_sb


def _emit_routed_body(nc, psum, t, pools, wsbs):
    wg_sb, wu_sb, wd_sb = wsbs
    xe_sb = pools["x"].tile([P, KH, NSLOT], bf16, tag="xe")
    for k in range(KH):
        nc.sync.dma_start(xe_sb[:, k], t["xe_r"][:, k, :NSLOT])

    chs = _chunks(NSLOT)                       # 4x512 + tail
    # single pass over all chunks: stationary reused across every chunk;
    # up reuses the gate psum banks after silu drains them (5 banks total,
    # leaving 3 for the down-proj pipeline)
    h_sb = pools["h"].tile([P, KI, NSLOT], bf16, tag="h", bufs=1)
    for m in range(KI):
        ps_gs = [psum.tile([P, w], f32, tag=f"g{ci}", bufs=1,
                           name=f"psg{ci}") for ci, (off, w) in enumerate(chs)]
        for k in range(KH):
            for ci, (off, w) in enumerate(chs):
                nc.tensor.matmul(
                    ps_gs[ci][:], wg_sb[:, k, m * P:(m + 1) * P],
                    xe_sb[:, k, off:off + w],
                    start=(k == 0), stop=(k == KH - 1))
        sg_l = []
        for ci, (off, w) in enumerate(chs):
            sg = pools["t"].tile([P, w], f32, tag=f"sg{ci}", bufs=2,
                                 name=f"sg{ci}")
            nc.scalar.activation(sg[:], ps_gs[ci][:], Silu)
            sg_l.append(sg)
        ps_us = [psum.tile([P, w], f32, tag=f"g{ci}", bufs=1,
                           name=f"psu{ci}") for ci, (off, w) in enumerate(chs)]
        for k in range(KH):
            for ci, (off, w) in enumerate(chs):
                nc.tensor.matmul(
                    ps_us[ci][:], wu_sb[:, k, m * P:(m + 1) * P],
                    xe_sb[:, k, off:off + w],
                    start=(k == 0), stop=(k == KH - 1))
        for ci, (off, w) in enumerate(chs):
            nc.vector.tensor_mul(out=h_sb[:, m, off:off + w],
                                 in0=sg_l[ci][:], in1=ps_us[ci][:])

    # ---- down-proj: weights stationary (reused across every chunk),
    # out [H-tile, tokens]; psum tags rotate over all 8 banks ----
    tags8 = ["g0", "g1", "g2", "g3", "g4", "d0", "d1", "d2"]
    nb = 0
    for hh in range(KI):
        ps_os = []
        for ci, (off, w) in enumerate(chs):
            ps_os.append(psum.tile([P, w], f32, tag=tags8[(nb + ci) % 8],
                                   bufs=1, name=f"psd{ci}"))
        nb += len(chs)
        for m in range(KI):
            for ci, (off, w) in enumerate(chs):
                nc.tensor.matmul(
                    ps_os[ci][:],
                    wd_sb[:, m, hh * P:(hh + 1) * P],
                    h_sb[:, m, off:off + w],
                    start=(m == 0), stop=(m == KI - 1))
        for ci, (off, w) in enumerate(chs):
            o_sb = pools["o"].tile([P, w], bf16, tag="o_sb")
            nc.vector.tensor_copy(o_sb[:], ps_os[ci][:])
            nc.sync.dma_start(
                t["eo"][hh * P:(hh + 1) * P, off:off + w], o_sb[:])


def _emit_shared_weights(nc, t, pools):
    """All shared-expert weights resident in SBUF (outside the timing loop —
    they are iteration-invariant, like the routed expert weights)."""
    w = pools["w"]
    wsg_sb = w.tile([P, KH, ISH], bf16, tag="wsg")
    wsu_sb = w.tile([P, KH, ISH], bf16, tag="wsu")
    for k in range(KH):
        nc.sync.dma_start(wsg_sb[:, k], t["wsg_r"][:, k])
        nc.sync.dma_start(wsu_sb[:, k], t["wsu_r"][:, k])
    wsd_blks = []
    for hn in range(H // FD):
        wsd_blk = pools["d"].tile([P, KISH, FD], bf16, tag="wsd")
        nc.sync.dma_start(wsd_blk[:], t["wsd_r"][:, :, hn * FD:(hn + 1) * FD])
        wsd_blks.append(wsd_blk)
    return wsg_sb, wsu_sb, wsd_blks


def _emit_shared_body(nc, psum, t, pools, wsbs):
    wsg_sb, wsu_sb, wsd_blks = wsbs
    xs_sb = pools["gu"].tile([P, KH, TSH], bf16, tag="xs")
    for k in range(KH):
        nc.sync.dma_start(xs_sb[:, k], t["xs_r"][:, k])
    hs_sb = pools["w"].tile([P, KISH, TSH], bf16, tag="hs")

    for m in range(KISH):
        ps_gs, ps_us = [], []
        for c2 in range(TSH // FD):            # 2 chunks of 512 tokens
            ps_gs.append(psum.tile([P, FD], f32, tag=f"g{c2}", bufs=1,
                                   name=f"psg{c2}"))
        for k in range(KH):
            for c2 in range(TSH // FD):
                nc.tensor.matmul(
                    ps_gs[c2][:], wsg_sb[:, k, m * P:(m + 1) * P],
                    xs_sb[:, k, c2 * FD:(c2 + 1) * FD],
                    start=(k == 0), stop=(k == KH - 1))
        for c2 in range(TSH // FD):
            ps_us.append(psum.tile([P, FD], f32, tag=f"g{2 + c2}", bufs=1,
                                   name=f"psu{c2}"))
        for k in range(KH):
            for c2 in range(TSH // FD):
                nc.tensor.matmul(
                    ps_us[c2][:], wsu_sb[:, k, m * P:(m + 1) * P],
                    xs_sb[:, k, c2 * FD:(c2 + 1) * FD],
                    start=(k == 0), stop=(k == KH - 1))
        for c2 in range(TSH // FD):
            sg = pools["t"].tile([P, FD], f32, tag="sg")
            nc.scalar.activation(sg[:], ps_gs[c2][:], Silu)
            nc.vector.tensor_mul(
                out=hs_sb[:, m, c2 * FD:(c2 + 1) * FD],
                in0=sg[:], in1=ps_us[c2][:])

    for tt in range(TSH // P):                 # 8 token tiles
        ps_os = [psum.tile([P, FD], f32, tag=f"d{(2 * tt + hn) % 3}", bufs=1,
                           name=f"pso{hn}")
                 for hn in range(H // FD)]
        for m in range(KISH):
            for hn in range(H // FD):
                nc.tensor.matmul(
                    ps_os[hn][:],
                    hs_sb[:, m, tt * P:(tt + 1) * P],
                    wsd_blks[hn][:, m],
                    start=(m == 0), stop=(m == KISH - 1))
        for hn in range(H // FD):
            o_sb = pools["o"].tile([P, FD], bf16, tag="o_sb")
            nc.vector.tensor_copy(o_sb[:], ps_os[hn][:])
            nc.sync.dma_start(
                t["so"][tt * P:(tt + 1) * P, hn * FD:(hn + 1) * FD], o_sb[:])


def _build_nc():
    nc = bacc.Bacc()
    t = _declare(nc)
    with tile.TileContext(nc) as tc:
        with tc.tile_pool(name="psum", bufs=1, space="PSUM") as psum:
            with contextlib.ExitStack() as rctx:
                pools = _pools_routed(tc, rctx)
                wsbs = _emit_routed_weights(nc, t, pools)
                _emit_routed_body(nc, psum, t, pools, wsbs)
            with contextlib.ExitStack() as sctx:
                pools = _pools_shared(tc, sctx)
                swsbs = _emit_shared_weights(nc, t, pools)
                _emit_shared_body(nc, psum, t, pools, swsbs)
    nc.compile()
    return nc


def _route(x, gate_w):
    """Router: mirrors the reference's jax ops (on CPU) for bit-exact top-k."""
    import jax
    import jax.numpy as jnp

    cpu = jax.devices("cpu")[0]
    with jax.default_device(cpu):
        logits = jnp.asarray(x).astype(jnp.float32) @ \
            jnp.asarray(gate_w).astype(jnp.float32).T
        scores = jax.nn.softmax(logits, axis=-1)
        topk_w, topk_idx = jax.lax.top_k(scores, TOPK)
        topk_w = topk_w / (topk_w.sum(-1, keepdims=True) + 1e-20) * RSF
        topk_w = np.asarray(topk_w)
        topk_idx = np.asarray(topk_idx)

    flat_e = topk_idx.reshape(-1).astype(np.int64)          # [N*K]
    onehot = (flat_e[:, None] == np.arange(E)[None, :]).astype(np.int32)
    pos = (np.cumsum(onehot, axis=0) - 1)[np.arange(flat_e.size), flat_e]
    keep = pos < CAP
    return topk_w, topk_idx, flat_e, pos, keep


def _prepare(hidden_states, gate_w, we_gate, we_up, we_down,
             ws_gate, ws_up, ws_down):
    import ml_dtypes
    BF = ml_dtypes.bfloat16

    x = np.asarray(hidden_states, np.float32).reshape(-1, H)
    topk_w, topk_idx, flat_e, pos, keep = _route(x, np.asarray(gate_w, np.float32))

    tok = np.repeat(np.arange(N), TOPK)
    e_s, p_s = flat_e[keep], pos[keep]
    n_s, w_s = tok[keep], topk_w.reshape(-1)[keep]

    # trim the dispatch buffer to the actual max expert load (padded to 128)
    counts = np.bincount(e_s, minlength=E)
    global NSLOT
    NSLOT = min(CAP, max(256, int(math.ceil(counts.max() / 128.0)) * 128))

    xbf = x.astype(BF)
    xe_all = np.zeros((E, H, CAP), BF)
    xe_all[e_s, :, p_s] = xbf[n_s]

    we_gate = np.asarray(we_gate, np.float32).astype(BF)
    we_up = np.asarray(we_up, np.float32).astype(BF)
    we_down = np.asarray(we_down, np.float32).astype(BF)
    wsg_t = np.ascontiguousarray(np.asarray(ws_gate, np.float32).astype(BF).T)
    wsu_t = np.ascontiguousarray(np.asarray(ws_up, np.float32).astype(BF).T)
    wsd_t = np.ascontiguousarray(np.asarray(ws_down, np.float32).astype(BF).T)

    in_maps = []
    for c in range(8):
        in_maps.append({
            "xe_t": np.ascontiguousarray(xe_all[c]),
            "wg_t": np.ascontiguousarray(we_gate[c].T),
            "wu_t": np.ascontiguousarray(we_up[c].T),
            "wd_t": np.ascontiguousarray(we_down[c].T),
            "xs_t": np.ascontiguousarray(xbf[c * TSH:(c + 1) * TSH].T),
            "wsg_t": wsg_t,
            "wsu_t": wsu_t,
            "wsd_t": wsd_t,
        })
    meta = (topk_idx, pos.reshape(N, TOPK), keep.reshape(N, TOPK), topk_w)
    return in_maps, meta


def _combine(results, meta, out_shape):
    topk_idx, pos2, keep2, topk_w = meta
    eo_all = np.stack([np.asarray(results[c]["eo"], np.float32)
                       for c in range(8)])                  # [E, H, CAP]
    y = np.concatenate([np.asarray(results[c]["so"], np.float32)
                        for c in range(8)], axis=0)         # [N, H]
    for k in range(TOPK):
        pk = np.clip(pos2[:, k], 0, CAP - 1)
        contrib = eo_all[topk_idx[:, k], :, pk]             # [N, H] unweighted
        contrib = contrib * topk_w[:, k:k + 1]
        y = y + np.where(keep2[:, k, None], contrib, np.float32(0.0))
    return y.reshape(out_shape).astype(np.float32)


def kernel(hidden_states, gate_w, we_gate, we_up, we_down,
           ws_gate, ws_up, ws_down):
    import time

    hidden_states = np.asarray(hidden_states, np.float32)
    in_maps, meta = _prepare(hidden_states, gate_w, we_gate, we_up, we_down,
                             ws_gate, ws_up, ws_down)
    nc = _build_nc()
    res = None
    for attempt in range(3):
        try:
            res = run_bass_kernel_spmd(nc, in_maps, list(range(8)))
            break
        except Exception:
            # Transient device wedges (NRT_EXEC_UNIT_UNRECOVERABLE) have been
            # observed through the axon tunnel; back off and retry.
            if attempt == 2:
                raise
            time.sleep(15)
    return _combine(res.results, meta, hidden_states.shape)


# revision 27
# speedup vs baseline: 1.3606x; 1.3606x over previous
"""MoE (routed top-2 + shared expert) Trainium2 kernel, 8-core expert-parallel.

Distribution strategy (hardcoded for B=4,S=2048,H=1024,E=8,K=2,I=1024,NSH=2):
 - Host computes the router (gate logits / softmax / top-2 / capacity mask)
   with the same jax-on-CPU ops as the reference, then dispatches tokens:
   core c receives the tokens routed to expert c, gathered and transposed to
   [H, cap] bf16, plus per-slot combine weights.
 - Core c runs expert c's SwiGLU MLP on its token buffer (weights resident in
   SBUF, bf16 matmuls into fp32 PSUM; down-proj emits eo as [H, slots]).
 - The shared expert is token-parallel: core c runs the full shared SwiGLU on
   tokens [c*1024, (c+1)*1024), with all shared weights SBUF-resident.
 - Host gathers the expert outputs (applying the top-2 combine weights during
   the gather) and adds the shared output.

bf16 matmuls: ~1.2-1.4x the fp32r MM rate (16-bit moving stream + weight-load
overlap), l2 rel err ~4.4e-3 vs the 2e-2 gate. The routed token buffer is
trimmed at build time to the actual max expert load (padded to 128): 2176
slots instead of the 2560 capacity for the reference routing, saving ~15% of
the routed compute.
"""

import contextlib
import math

import numpy as np

import concourse.mybir as mybir
import concourse.tile as tile
from concourse import bacc
from concourse.bass_utils import run_bass_kernel_spmd

# Problem dims (hardcoded per spec)
B, S, H = 4, 2048, 1024
E, TOPK, I = 8, 2, 1024
NSH = 2
ISH = NSH * I            # 2048 shared intermediate
RSF = 1.0
N = B * S                # 8192 tokens
CAP = 2560               # ceil(1.25 * N * TOPK / E)
TSH = N // 8             # shared-expert tokens per core
P = 128
f32 = mybir.dt.float32
bf16 = mybir.dt.bfloat16
KH = H // P              # 8 contraction subtiles over H
KI = I // P              # 8 over I
KISH = ISH // P          # 16 over ISH
FD = 512                 # matmul moving free dim
Silu = mybir.ActivationFunctionType.Silu

# Active routed slots (padded to 128); set by kernel() from the actual max
# expert load before building. 2176 covers the near-uniform load of the
# reference distribution (max count ~2080); kernel() adjusts it at runtime.
NSLOT = 2176


def _chunks(total):
    """[(offset, width)] with width 512 except a possibly-smaller tail."""
    out = []
    off = 0
    while off < total:
        w = min(FD, total - off)
        out.append((off, w))
        off += w
    return out


def _groups(total):
    """Chunk groups of <=4 chunks (psum: 4 gate + 4 up banks)."""
    chs = _chunks(total)
    return [chs[i:i + 4] for i in range(0, len(chs), 4)]


def _declare(nc):
    t = {}
    t["xe_t"] = nc.dram_tensor("xe_t", [H, CAP], bf16, kind="ExternalInput")
    t["wg_t"] = nc.dram_tensor("wg_t", [H, I], bf16, kind="ExternalInput")
    t["wu_t"] = nc.dram_tensor("wu_t", [H, I], bf16, kind="ExternalInput")
    t["wd_t"] = nc.dram_tensor("wd_t", [I, H], bf16, kind="ExternalInput")
    t["xs_t"] = nc.dram_tensor("xs_t", [H, TSH], bf16, kind="ExternalInput")
    t["wsg_t"] = nc.dram_tensor("wsg_t", [H, ISH], bf16, kind="ExternalInput")
    t["wsu_t"] = nc.dram_tensor("wsu_t", [H, ISH], bf16, kind="ExternalInput")
    t["wsd_t"] = nc.dram_tensor("wsd_t", [ISH, H], bf16, kind="ExternalInput")
    t["eo"] = nc.dram_tensor("eo", [H, CAP], bf16, kind="ExternalOutput")
    t["so"] = nc.dram_tensor("so", [TSH, H], bf16, kind="ExternalOutput")
    _rearranges(t)
    return t


def _declare_internal(nc):
    """Same tensors as _declare but Internal DRAM — used by timing harnesses
    so per-call wall time carries no host<->device transfer of real data."""
    t = {}
    for name, shape, dt in [
            ("xe_t", [H, CAP], bf16), ("wg_t", [H, I], bf16),
            ("wu_t", [H, I], bf16), ("wd_t", [I, H], bf16),
            ("xs_t", [H, TSH], bf16),
            ("wsg_t", [H, ISH], bf16), ("wsu_t", [H, ISH], bf16),
            ("wsd_t", [ISH, H], bf16), ("eo", [H, CAP], bf16),
            ("so", [TSH, H], bf16)]:
        t[name] = nc.dram_tensor(name, shape, dt)
    _rearranges(t)
    return t


def _rearranges(t):
    t["xe_r"] = t["xe_t"][:].rearrange("(k p) t -> p k t", p=P)    # [128,8,2560]
    t["wg_r"] = t["wg_t"][:].rearrange("(k p) i -> p k i", p=P)    # [128,8,1024]
    t["wu_r"] = t["wu_t"][:].rearrange("(k p) i -> p k i", p=P)
    t["wd_r"] = t["wd_t"][:].rearrange("(k p) h -> p k h", p=P)
    t["xs_r"] = t["xs_t"][:].rearrange("(k p) t -> p k t", p=P)    # [128,8,1024]
    t["wsg_r"] = t["wsg_t"][:].rearrange("(k p) i -> p k i", p=P)  # [128,8,2048]
    t["wsu_r"] = t["wsu_t"][:].rearrange("(k p) i -> p k i", p=P)
    t["wsd_r"] = t["wsd_t"][:].rearrange("(k p) h -> p k h", p=P)  # [128,16,1024]


def _pools_routed(tc, ctx):
    return {
        "w": ctx.enter_context(tc.tile_pool(name="wR", bufs=1)),
        "x": ctx.enter_context(tc.tile_pool(name="xR", bufs=2)),
        "h": ctx.enter_context(tc.tile_pool(name="hR", bufs=2)),
        "t": ctx.enter_context(tc.tile_pool(name="tR", bufs=3)),
        "o": ctx.enter_context(tc.tile_pool(name="oR", bufs=4)),
    }


def _pools_shared(tc, ctx):
    return {
        "w": ctx.enter_context(tc.tile_pool(name="wS", bufs=1)),
        "gu": ctx.enter_context(tc.tile_pool(name="guS", bufs=2)),
        "d": ctx.enter_context(tc.tile_pool(name="dS", bufs=2)),
        "t": ctx.enter_context(tc.tile_pool(name="tS", bufs=3)),
        "o": ctx.enter_context(tc.tile_pool(name="oS", bufs=4)),
    }


def _emit_routed_weights(nc, t, pools):
    """Load expert weights resident in SBUF (once, outside any timing loop)."""
    w = pools["w"]
    wg_sb = w.tile([P, KH, I], bf16, tag="wg")
    wu_sb = w.tile([P, KH, I], bf16, tag="wu")
    wd_sb = w.tile([P, KI, H], bf16, tag="wd")
    for k in range(KH):
        nc.sync.dma_start(wg_sb[:, k], t["wg_r"][:, k])
        nc.sync.dma_start(wu_sb[:, k], t["wu_r"][:, k])
    for k in range(KI):
        nc.sync.dma_start(wd_sb[:, k], t["wd_r"][:, k])
    return wg_sb, wu_sb, wd_sb


def _emit_routed_body(nc, psum, t, pools, wsbs):
    wg_sb, wu_sb, wd_sb = wsbs
    xe_sb = pools["x"].tile([P, KH, NSLOT], bf16, tag="xe")
    for k in range(KH):
        nc.sync.dma_start(xe_sb[:, k], t["xe_r"][:, k, :NSLOT])

    chs = _chunks(NSLOT)                       # 4x512 + tail
    # single pass over all chunks: stationary reused across every chunk;
    # up reuses the gate psum banks after silu drains them (5 banks total,
    # leaving 3 for the down-proj pipeline)
    h_sb = pools["h"].tile([P, KI, NSLOT], bf16, tag="h", bufs=1)
    for m in range(KI):
        ps_gs = [psum.tile([P, w], f32, tag=f"g{ci}", bufs=1,
                           name=f"psg{ci}") for ci, (off, w) in enumerate(chs)]
        for k in range(KH):
            for ci, (off, w) in enumerate(chs):
                nc.tensor.matmul(
                    ps_gs[ci][:], wg_sb[:, k, m * P:(m + 1) * P],
                    xe_sb[:, k, off:off + w],
                    start=(k == 0), stop=(k == KH - 1))
        sg_l = []
        for ci, (off, w) in enumerate(chs):
            sg = pools["t"].tile([P, w], f32, tag=f"sg{ci}", bufs=2,
                                 name=f"sg{ci}")
            nc.scalar.activation(sg[:], ps_gs[ci][:], Silu)
            sg_l.append(sg)
        ps_us = [psum.tile([P, w], f32, tag=f"g{ci}", bufs=1,
                           name=f"psu{ci}") for ci, (off, w) in enumerate(chs)]
        for k in range(KH):
            for ci, (off, w) in enumerate(chs):
                nc.tensor.matmul(
                    ps_us[ci][:], wu_sb[:, k, m * P:(m + 1) * P],
                    xe_sb[:, k, off:off + w],
                    start=(k == 0), stop=(k == KH - 1))
        for ci, (off, w) in enumerate(chs):
            nc.vector.tensor_mul(out=h_sb[:, m, off:off + w],
                                 in0=sg_l[ci][:], in1=ps_us[ci][:])

    # ---- down-proj: weights stationary (reused across every chunk),
    # out [H-tile, tokens]; psum tags rotate over all 8 banks ----
    tags8 = ["g0", "g1", "g2", "g3", "g4", "d0", "d1", "d2"]
    nb = 0
    for hh in range(KI):
        ps_os = []
        for ci, (off, w) in enumerate(chs):
            ps_os.append(psum.tile([P, w], f32, tag=tags8[(nb + ci) % 8],
                                   bufs=1, name=f"psd{ci}"))
        nb += len(chs)
        for m in range(KI):
            for ci, (off, w) in enumerate(chs):
                nc.tensor.matmul(
                    ps_os[ci][:],
                    wd_sb[:, m, hh * P:(hh + 1) * P],
                    h_sb[:, m, off:off + w],
                    start=(m == 0), stop=(m == KI - 1))
        for ci, (off, w) in enumerate(chs):
            o_sb = pools["o"].tile([P, w], bf16, tag="o_sb")
            nc.vector.tensor_copy(o_sb[:], ps_os[ci][:])
            nc.sync.dma_start(
                t["eo"][hh * P:(hh + 1) * P, off:off + w], o_sb[:])


def _emit_shared_weights(nc, t, pools):
    """All shared-expert weights resident in SBUF (outside the timing loop —
    they are iteration-invariant, like the routed expert weights)."""
    w = pools["w"]
    wsg_sb = w.tile([P, KH, ISH], bf16, tag="wsg")
    wsu_sb = w.tile([P, KH, ISH], bf16, tag="wsu")
    for k in range(KH):
        nc.sync.dma_start(wsg_sb[:, k], t["wsg_r"][:, k])
        nc.sync.dma_start(wsu_sb[:, k], t["wsu_r"][:, k])
    wsd_blks = []
    for hn in range(H // FD):
        wsd_blk = pools["d"].tile([P, KISH, FD], bf16, tag="wsd")
        nc.sync.dma_start(wsd_blk[:], t["wsd_r"][:, :, hn * FD:(hn + 1) * FD])
        wsd_blks.append(wsd_blk)
    return wsg_sb, wsu_sb, wsd_blks


def _emit_shared_body(nc, psum, t, pools, wsbs):
    wsg_sb, wsu_sb, wsd_blks = wsbs
    xs_sb = pools["gu"].tile([P, KH, TSH], bf16, tag="xs")
    for k in range(KH):
        nc.sync.dma_start(xs_sb[:, k], t["xs_r"][:, k])
    hs_sb = pools["w"].tile([P, KISH, TSH], bf16, tag="hs")

    # gate/up with split-K: each 8-deep contraction runs as two 4-deep psum
    # chains (overwrite-mode start=True matmuls stream ~2x faster than
    # accumulating ones). A TensorTensor op may read only ONE input from
    # PSUM (NCC_IBVF027) and only Act/DVE have PSUM ports, so one half is
    # copied to SBUF on the Act engine before the DVE add.
    KHH = KH // 2
    for m in range(KISH):
        ps_ga, ps_gb = [], []
        for c2 in range(TSH // FD):            # 2 chunks of 512 tokens
            ps_ga.append(psum.tile([P, FD], f32, tag=f"g{c2}", bufs=1,
                                   name=f"psga{c2}"))
            ps_gb.append(psum.tile([P, FD], f32, tag=f"g{2 + c2}", bufs=1,
                                   name=f"psgb{c2}"))
        for k in range(KH):
            half = ps_ga if k < KHH else ps_gb
            for c2 in range(TSH // FD):
                nc.tensor.matmul(
                    half[c2][:], wsg_sb[:, k, m * P:(m + 1) * P],
                    xs_sb[:, k, c2 * FD:(c2 + 1) * FD],
                    start=(k % KHH == 0), stop=(k % KHH == KHH - 1))
        sgs = []
        for c2 in range(TSH // FD):
            gac = pools["t"].tile([P, FD], f32, tag=f"gc{c2}", bufs=2,
                                  name=f"gac{c2}")
            nc.scalar.copy(gac[:], ps_ga[c2][:])
            tg = pools["t"].tile([P, FD], f32, tag=f"tg{c2}", bufs=2,
                                 name=f"tg{c2}")
            nc.vector.tensor_add(out=tg[:], in0=ps_gb[c2][:], in1=gac[:])
            sg = pools["t"].tile([P, FD], f32, tag=f"sg{c2}", bufs=2,
                                 name=f"sg{c2}")
            nc.scalar.activation(sg[:], tg[:], Silu)
            sgs.append(sg)
        ps_ua, ps_ub = [], []
        for c2 in range(TSH // FD):
            ps_ua.append(psum.tile([P, FD], f32, tag=f"g{c2}", bufs=1,
                                   name=f"psua{c2}"))
            ps_ub.append(psum.tile([P, FD], f32, tag=f"g{2 + c2}", bufs=1,
                                   name=f"psub{c2}"))
        for k in range(KH):
            half = ps_ua if k < KHH else ps_ub
            for c2 in range(TSH // FD):
                nc.tensor.matmul(
                    half[c2][:], wsu_sb[:, k, m * P:(m + 1) * P],
                    xs_sb[:, k, c2 * FD:(c2 + 1) * FD],
                    start=(k % KHH == 0), stop=(k % KHH == KHH - 1))
        for c2 in range(TSH // FD):
            uac = pools["t"].tile([P, FD], f32, tag=f"uc{c2}", bufs=2,
                                  name=f"uac{c2}")
            nc.scalar.copy(uac[:], ps_ua[c2][:])
            tu = pools["t"].tile([P, FD], f32, tag=f"tu{c2}", bufs=2,
                                 name=f"tu{c2}")
            nc.vector.tensor_add(out=tu[:], in0=ps_ub[c2][:], in1=uac[:])
            nc.vector.tensor_mul(
                out=hs_sb[:, m, c2 * FD:(c2 + 1) * FD],
                in0=sgs[c2][:], in1=tu[:])

    for tt in range(TSH // P):                 # 8 token tiles
        ps_os = [psum.tile([P, FD], f32, tag=f"d{(2 * tt + hn) % 3}", bufs=1,
                           name=f"pso{hn}")
                 for hn in range(H // FD)]
        for m in range(KISH):
            for hn in range(H // FD):
                nc.tensor.matmul(
                    ps_os[hn][:],
                    hs_sb[:, m, tt * P:(tt + 1) * P],
                    wsd_blks[hn][:, m],
                    start=(m == 0), stop=(m == KISH - 1))
        for hn in range(H // FD):
            o_sb = pools["o"].tile([P, FD], bf16, tag="o_sb")
            nc.vector.tensor_copy(o_sb[:], ps_os[hn][:])
            nc.sync.dma_start(
                t["so"][tt * P:(tt + 1) * P, hn * FD:(hn + 1) * FD], o_sb[:])


def _build_nc():
    nc = bacc.Bacc()
    t = _declare(nc)
    with tile.TileContext(nc) as tc:
        with tc.tile_pool(name="psum", bufs=1, space="PSUM") as psum:
            with contextlib.ExitStack() as rctx:
                pools = _pools_routed(tc, rctx)
                wsbs = _emit_routed_weights(nc, t, pools)
                _emit_routed_body(nc, psum, t, pools, wsbs)
            with contextlib.ExitStack() as sctx:
                pools = _pools_shared(tc, sctx)
                swsbs = _emit_shared_weights(nc, t, pools)
                _emit_shared_body(nc, psum, t, pools, swsbs)
    nc.compile()
    return nc


def _route(x, gate_w):
    """Router: mirrors the reference's jax ops (on CPU) for bit-exact top-k."""
    import jax
    import jax.numpy as jnp

    cpu = jax.devices("cpu")[0]
    with jax.default_device(cpu):
        logits = jnp.asarray(x).astype(jnp.float32) @ \
            jnp.asarray(gate_w).astype(jnp.float32).T
        scores = jax.nn.softmax(logits, axis=-1)
        topk_w, topk_idx = jax.lax.top_k(scores, TOPK)
        topk_w = topk_w / (topk_w.sum(-1, keepdims=True) + 1e-20) * RSF
        topk_w = np.asarray(topk_w)
        topk_idx = np.asarray(topk_idx)

    flat_e = topk_idx.reshape(-1).astype(np.int64)          # [N*K]
    onehot = (flat_e[:, None] == np.arange(E)[None, :]).astype(np.int32)
    pos = (np.cumsum(onehot, axis=0) - 1)[np.arange(flat_e.size), flat_e]
    keep = pos < CAP
    return topk_w, topk_idx, flat_e, pos, keep


def _prepare(hidden_states, gate_w, we_gate, we_up, we_down,
             ws_gate, ws_up, ws_down):
    import ml_dtypes
    BF = ml_dtypes.bfloat16

    x = np.asarray(hidden_states, np.float32).reshape(-1, H)
    topk_w, topk_idx, flat_e, pos, keep = _route(x, np.asarray(gate_w, np.float32))

    tok = np.repeat(np.arange(N), TOPK)
    e_s, p_s = flat_e[keep], pos[keep]
    n_s, w_s = tok[keep], topk_w.reshape(-1)[keep]

    # trim the dispatch buffer to the actual max expert load (padded to 128)
    counts = np.bincount(e_s, minlength=E)
    global NSLOT
    NSLOT = min(CAP, max(256, int(math.ceil(counts.max() / 128.0)) * 128))

    xbf = x.astype(BF)
    xe_all = np.zeros((E, H, CAP), BF)
    xe_all[e_s, :, p_s] = xbf[n_s]

    we_gate = np.asarray(we_gate, np.float32).astype(BF)
    we_up = np.asarray(we_up, np.float32).astype(BF)
    we_down = np.asarray(we_down, np.float32).astype(BF)
    wsg_t = np.ascontiguousarray(np.asarray(ws_gate, np.float32).astype(BF).T)
    wsu_t = np.ascontiguousarray(np.asarray(ws_up, np.float32).astype(BF).T)
    wsd_t = np.ascontiguousarray(np.asarray(ws_down, np.float32).astype(BF).T)

    in_maps = []
    for c in range(8):
        in_maps.append({
            "xe_t": np.ascontiguousarray(xe_all[c]),
            "wg_t": np.ascontiguousarray(we_gate[c].T),
            "wu_t": np.ascontiguousarray(we_up[c].T),
            "wd_t": np.ascontiguousarray(we_down[c].T),
            "xs_t": np.ascontiguousarray(xbf[c * TSH:(c + 1) * TSH].T),
            "wsg_t": wsg_t,
            "wsu_t": wsu_t,
            "wsd_t": wsd_t,
        })
    meta = (topk_idx, pos.reshape(N, TOPK), keep.reshape(N, TOPK), topk_w)
    return in_maps, meta


def _combine(results, meta, out_shape):
    topk_idx, pos2, keep2, topk_w = meta
    eo_all = np.stack([np.asarray(results[c]["eo"], np.float32)
                       for c in range(8)])                  # [E, H, CAP]
    y = np.concatenate([np.asarray(results[c]["so"], np.float32)
                        for c in range(8)], axis=0)         # [N, H]
    for k in range(TOPK):
        pk = np.clip(pos2[:, k], 0, CAP - 1)
        contrib = eo_all[topk_idx[:, k], :, pk]             # [N, H] unweighted
        contrib = contrib * topk_w[:, k:k + 1]
        y = y + np.where(keep2[:, k, None], contrib, np.float32(0.0))
    return y.reshape(out_shape).astype(np.float32)


def kernel(hidden_states, gate_w, we_gate, we_up, we_down,
           ws_gate, ws_up, ws_down):
    import time

    hidden_states = np.asarray(hidden_states, np.float32)
    in_maps, meta = _prepare(hidden_states, gate_w, we_gate, we_up, we_down,
                             ws_gate, ws_up, ws_down)
    nc = _build_nc()
    res = None
    for attempt in range(3):
        try:
            res = run_bass_kernel_spmd(nc, in_maps, list(range(8)))
            break
        except Exception:
            # Transient device wedges (NRT_EXEC_UNIT_UNRECOVERABLE) have been
            # observed through the axon tunnel; back off and retry.
            if attempt == 2:
                raise
            time.sleep(15)
    return _combine(res.results, meta, hidden_states.shape)
